# revision 19
# baseline (speedup 1.0000x reference)
"""Trainium2 Bass kernel for nn_MemBlock (dense transformer block).

Reference computation (B=4, T=1024, H=1024, K=16 heads, hd=64):
    h  = LN(x);  q,k,v = h@Wq, h@Wk, h@Wv  (per-head split)
    s  = q k^T / sqrt(hd);  masked (future) positions FILLED with 1e-9 (not -inf)
    a  = softmax(s);  y = a v;  x = x + y
    h2 = LN(x);  out = x + gelu(h2@W1)@W2

Key numerical fact exploited: in fp32, exp(1e-9) == 1.0 exactly, so every
"masked" (strictly-future) position carries softmax weight exp(0)=1.  A fully
masked 128x128 score block contributes plain column-sums of V to the
numerator and a count to the denominator -- folded into the attention-value
accumulation as one extra tiny matmul (suffix^T x block-indicator) per
(head, column-half).  Only lower-triangular blocks of the score matrix are
computed; the diagonal block is repaired multiplicatively on the exp output
(e = e*tri + (1-tri), on the otherwise-idle gpsimd engine) so masked entries
carry weight exactly 1, matching the reference.

Sharding (8 cores, SPMD): core c handles batch b=c//2 and head-half h=c%2:
attention over heads [8h, 8h+8) for ALL T rows, then a pairwise exchange
hands each core its own T-row half of the full-width attention output, and
each core runs LN2 + the full-weight MLP on its 512 own rows.

v7 schedule notes:
  - Engine queues are in-order, so every op is pinned to an engine chosen so
    that no queue ever head-of-line-blocks on a slower producer: scalar does
    ONLY exps during attention; vector does masks/finalize/copies; gpsimd
    (idle otherwise) repairs the diagonal blocks; the next pair's q/k fp8
    DoubleRow chains are drip-fed INTO the score/AV loop one matmul at a
    time so the PE never builds a wall in front of the next score block.
  - LN1 is software-pipelined one full stage (stats of tile t+1 issue before
    the rstd/normalize of tile t) to hide the vector->scalar->vector
    round-trip; dummy "warm" matmuls keep the PE HAM clock at 2.4 GHz
    through the PE-sparse LN1 and exchange windows.
  - The y exchange is 4 fp8 pieces (one per head pair) ReduceScatter'd as
    soon as each pair finishes; receives/adds/LN2-stats run strictly after
    the attention loop so no engine queue waits on a collective mid-flight.
  - q/k/v projections are fp8e4 DoubleRow (weights prescaled x16 on the
    host, 1/16 folded into the PSUM->SBUF copies); scores/AV stay bf16.
"""

import numpy as np
import ml_dtypes

import concourse.bass as bass
import concourse.tile as tile
from concourse import bacc, mybir
from concourse.bass_utils import run_bass_kernel_spmd
from concourse.masks import make_identity, make_upper_triangular

F32 = mybir.dt.float32
BF16 = mybir.dt.bfloat16
FP8 = mybir.dt.float8e4
AF = mybir.ActivationFunctionType
ALU = mybir.AluOpType
DR = mybir.MatmulPerfMode.DoubleRow

B, T, H, NK, HD = 4, 1024, 1024, 16, 64
NHC = 8          # heads per core
TO = 512         # own rows per core
FF = 4 * H       # 4096
P = 128
EPS = 1e-5
WS = 16.0        # fp8 weight prescale

REPLICA_GROUPS = [[0, 1], [2, 3], [4, 5], [6, 7]]

_CACHE = {}


def _build_program():
    nc = bacc.Bacc("TRN2", target_bir_lowering=False, debug=False, num_devices=8)

    x_full = nc.dram_tensor("x_full", [T, H], F32, kind="ExternalInput").ap()
    x_own = nc.dram_tensor("x_own", [TO, H], F32, kind="ExternalInput").ap()
    wq = nc.dram_tensor("wq", [H, NHC * HD], FP8, kind="ExternalInput").ap()
    wk = nc.dram_tensor("wk", [H, NHC * HD], FP8, kind="ExternalInput").ap()
    wv = nc.dram_tensor("wv", [H, NHC * HD], FP8, kind="ExternalInput").ap()
    w1 = nc.dram_tensor("w1", [H, FF], BF16, kind="ExternalInput").ap()
    w2 = nc.dram_tensor("w2", [FF, H], BF16, kind="ExternalInput").ap()
    sel = nc.dram_tensor("sel", [1, 2], F32, kind="ExternalInput").ap()
    bind = nc.dram_tensor("bind", [8, T], BF16, kind="ExternalInput").ap()
    out = nc.dram_tensor("out", [TO, H], F32, kind="ExternalOutput").ap()

    cc_wu_in = nc.dram_tensor("cc_wu_in", [2, 16], BF16)
    cc_wu_out = nc.dram_tensor("cc_wu_out", [1, 16], BF16)
    cc_in = [nc.dram_tensor(f"cc_in{p}", [2, TO, 256], FP8) for p in range(4)]
    cc_out = [nc.dram_tensor(f"cc_out{p}", [TO, 256], FP8) for p in range(4)]

    with tile.TileContext(nc) as tc:
        with tc.tile_pool(name="consts", bufs=1) as consts, \
             tc.tile_pool(name="persist", bufs=1) as persist, \
             tc.tile_pool(name="w1pool", bufs=2) as w1pool:

            ident = consts.tile([P, P], F32)
            make_identity(nc, ident)
            tri = consts.tile([P, P], BF16)   # tri[p,t] = 1 if p <= t else 0
            make_upper_triangular(nc, tri, val=1.0, diag=True)
            itri = consts.tile([P, P], BF16)  # 1 - tri (strictly-lower)
            nc.vector.memset(itri, 1.0)
            nc.vector.tensor_tensor(itri, itri, tri, op=ALU.subtract)
            eps_t = consts.tile([P, 1], F32)
            nc.vector.memset(eps_t, EPS)
            # ind[p, i, j] = 1 if i > j else 0 (suffix-of-blocks indicator)
            ind = consts.tile([P, 8, 8], BF16)
            nc.vector.memset(ind, 0.0)
            for i in range(1, 8):
                nc.vector.memset(ind[:, i, 0:i], 1.0)
            # blockind[j, q] = 1 if q//128 == j (suffix broadcast MM rhs)
            blockind = consts.tile([8, T], BF16)
            nc.gpsimd.dma_start(out=blockind, in_=bind)
            dmy = consts.tile([P, 512], BF16)
            nc.vector.memset(dmy, 1.0)
            sel_sb = consts.tile([P, 2], F32)
            nc.gpsimd.dma_start(
                out=sel_sb,
                in_=bass.AP(tensor=sel.tensor, offset=0, ap=[[0, P], [1, 2]]),
            )

            x_own_sb = persist.tile([P, 4, H], F32)  # becomes z = x+y, then out
            w2_sb = persist.tile([P, 32, H], BF16)
            stats2 = persist.tile([P, 4, 2, 6], F32)

            with tc.tile_pool(name="attn_big", bufs=1) as big, \
                 tc.tile_pool(name="epool", bufs=4) as epool, \
                 tc.tile_pool(name="small", bufs=2) as small, \
                 tc.tile_pool(name="stgpool", bufs=2) as stgpool, \
                 tc.tile_pool(name="zpool", bufs=4) as zpool, \
                 tc.tile_pool(name="ln", bufs=3) as ln, \
                 tc.tile_pool(name="ps6", bufs=4, space="PSUM") as ps6, \
                 tc.tile_pool(name="ps_yaug", bufs=2, space="PSUM") as ps_yaug, \
                 tc.tile_pool(name="ps_qkv", bufs=2, space="PSUM") as ps_qkv:

                hT8 = big.tile([P, 8, T], FP8)
                qT = big.tile([P, 4, T], BF16)
                kT = big.tile([P, 4, T], BF16)
                v_aug = big.tile([P, 8, NHC, HD + 1], BF16)
                wq_sb = big.tile([P, 8, NHC * HD], FP8)
                wk_sb = big.tile([P, 8, NHC * HD], FP8)
                wv_sb = big.tile([P, 8, NHC * HD], FP8)
                nc.gpsimd.dma_start(out=wv_sb, in_=wv.rearrange("(o p) j -> p o j", p=P))
                nc.gpsimd.dma_start(out=wq_sb, in_=wq.rearrange("(o p) j -> p o j", p=P))
                nc.gpsimd.dma_start(out=wk_sb, in_=wk.rearrange("(o p) j -> p o j", p=P))
                nc.gpsimd.dma_start(x_own_sb, x_own.rearrange("(o p) f -> p o f", p=P))
                # dummy collective to absorb the CC-stream warmup delay
                nc.gpsimd.collective_compute(
                    "ReduceScatter", ALU.add,
                    ins=[cc_wu_in[:]], outs=[cc_wu_out[:]],
                    replica_groups=REPLICA_GROUPS,
                )

                def warm(nmm):
                    """No-dep matmuls keeping the PE HAM clock at 2.4 GHz
                    through PE-sparse stretches."""
                    wp = ps6.tile([P, 512], F32, tag="ps", name="warm")
                    for _ in range(nmm):
                        nc.tensor.matmul(
                            wp, lhsT=dmy[:, 0:P], rhs=dmy,
                            start=True, stop=True, skip_group_check=True,
                        )

                warm(12)

                # ---- Phase 1: LN1, software-pipelined one stage ----
                n_sc_cp = 6  # hT8 cast-copies on scalar (rest on vector)

                def ln1_a(tt, st):
                    xt = ln.tile([P, H], F32, tag="xt", name="xt")
                    st['xt'] = xt
                    nc.sync.dma_start(xt, x_full[tt * P:(tt + 1) * P, :])
                    stats = ln.tile([P, 2, 6], F32, tag="stats", name="stats")
                    st['stats'] = stats
                    nc.vector.bn_stats(stats[:, 0, :], xt[:, 0:512])
                    nc.vector.bn_stats(stats[:, 1, :], xt[:, 512:1024])
                    mv = ln.tile([P, 2], F32, tag="mv", name="mv")
                    st['mv'] = mv
                    nc.vector.bn_aggr(mv, stats)
                    sq = ln.tile([P, 1], F32, tag="sq", name="sq")
                    st['sq'] = sq
                    nc.scalar.activation(sq, mv[:, 1:2], AF.Sqrt,
                                         bias=eps_t[:, 0:1])

                def ln1_b(tt, st):
                    rstd = ln.tile([P, 1], F32, tag="rstd")
                    nc.vector.reciprocal(rstd, st['sq'])
                    h = ln.tile([P, H], F32, tag="h")
                    nc.vector.tensor_scalar(
                        h, st['xt'], st['mv'][:, 0:1], rstd,
                        ALU.subtract, ALU.mult,
                    )
                    for hi in range(8):
                        pt = ps6.tile([P, 512], F32, tag="ps", name="trp")[:, 0:P]
                        nc.tensor.transpose(pt, h[:, hi * P:(hi + 1) * P], ident)
                        if hi < n_sc_cp:
                            nc.scalar.copy(
                                out=hT8[:, hi, tt * P:(tt + 1) * P], in_=pt)
                        else:
                            nc.vector.tensor_copy(
                                out=hT8[:, hi, tt * P:(tt + 1) * P], in_=pt)
                    ps = ps_qkv.tile([P, 512], F32, tag="qkv")
                    for kk in range(4):
                        nc.tensor.matmul(
                            ps,
                            lhsT=hT8[:, 2 * kk:2 * kk + 2, tt * P:(tt + 1) * P],
                            rhs=wv_sb[:, 2 * kk:2 * kk + 2, :],
                            start=(kk == 0), stop=(kk == 3), perf_mode=DR,
                        )
                    nc.scalar.mul(
                        v_aug[:, tt, :, 0:HD],
                        ps.rearrange("p (h d) -> p h d", h=NHC),
                        1.0 / WS,
                    )
                    warm(5)

                sts = [dict() for _ in range(8)]
                ln1_a(0, sts[0])
                for tt in range(8):
                    if tt < 7:
                        ln1_a(tt + 1, sts[tt + 1])
                    ln1_b(tt, sts[tt])
                nc.vector.memset(v_aug[:, :, :, HD:HD + 1], 1.0)

                # Early weight DMAs on the scalar queue; late w1 chunks go
                # just-in-time on sync during MLP1.
                w1c = [w1pool.tile([P, 8, 512], BF16, tag="w1c", name=f"w1c{i}")
                       for i in range(8)]

                def w1_dma(eng, i):
                    eng.dma_start(
                        w1c[i],
                        w1[:, i * 512:(i + 1) * 512].rearrange(
                            "(o p) f -> p o f", p=P),
                    )

                w1_dma(nc.scalar, 0)
                w1_dma(nc.scalar, 1)
                nc.scalar.dma_start(w2_sb, w2.rearrange("(o p) n -> p o n", p=P))

                def qk_steps(jt, ch):
                    """One-matmul-at-a-time issue steps for pair jt's q/k
                    DoubleRow chains over T-column half ch."""
                    tiles = {}
                    steps = []

                    def mk(dst, w_sb, kk, name):
                        def f():
                            if kk == 0:
                                tiles[name] = ps_qkv.tile(
                                    [P, 512], F32, tag="qkv", name=name)
                            nc.tensor.matmul(
                                tiles[name],
                                lhsT=w_sb[:, 2 * kk:2 * kk + 2,
                                          jt * P:(jt + 1) * P],
                                rhs=hT8[:, 2 * kk:2 * kk + 2,
                                        ch * 512:(ch + 1) * 512],
                                start=(kk == 0), stop=(kk == 3), perf_mode=DR,
                            )
                            if kk == 3:
                                nc.vector.tensor_scalar_mul(
                                    dst[:, jt, ch * 512:(ch + 1) * 512],
                                    tiles[name], 1.0 / WS)
                        return f

                    for kk in range(4):
                        steps.append(mk(qT, wq_sb, kk, "psq"))
                        steps.append(mk(kT, wk_sb, kk, "psk"))
                    return steps

                for st in qk_steps(0, 0) + qk_steps(0, 1):
                    st()

                # ---- Phase 2: attention per head pair ----
                for jt in range(4):
                    pair = (2 * jt, 2 * jt + 1)

                    # sufT[j, d] = sum_{i>j} colsum(V_aug_i)[d]: [8, 65]/head
                    sufp = ps6.tile([P, 512], F32, tag="ps", name="sufp")
                    for z, h_ in enumerate(pair):
                        for i in range(1, 8):
                            nc.tensor.matmul(
                                sufp[0:8, 65 * z:65 * z + 65],
                                lhsT=ind[:, i, :],
                                rhs=v_aug[:, i, h_, :],
                                start=(i == 1), stop=(i == 7),
                                skip_group_check=True,
                            )
                    sufT_sb = small.tile([8, 130], BF16, tag="sufT")
                    nc.vector.tensor_copy(out=sufT_sb, in_=sufp[0:8, 0:130])

                    stg = stgpool.tile([P, 8, P], BF16, tag="stg")
                    stg2 = stgpool.tile([P, 8, 256], FP8, tag="stg2")

                    for c in range(2):
                        qsteps = qk_steps(jt + 1, c) if jt < 3 else []
                        yaugs = [
                            ps_yaug.tile([HD + 1, 512], F32, tag="yaug",
                                         name=f"yaug{z}")
                            for z in range(2)
                        ]
                        for z in range(2):
                            nc.tensor.matmul(
                                yaugs[z],
                                lhsT=sufT_sb[0:8, 65 * z:65 * z + 65],
                                rhs=blockind[0:8, 512 * c:512 * (c + 1)],
                                start=True, stop=False, skip_group_check=True,
                            )
                        # non-diagonal key blocks first: their exps flow with
                        # no repair op in the score->exp->AV chain
                        ilist = [i for i in range(8)
                                 if 512 * (c + 1) - 128 * i > 0]
                        ilist = ([i for i in ilist
                                  if not 4 * c <= i <= 4 * c + 3]
                                 + [i for i in ilist if 4 * c <= i <= 4 * c + 3])
                        nq = ((len(qsteps) + len(ilist) - 1) // len(ilist)
                              if qsteps else 0)
                        for idx, i in enumerate(ilist):
                            sc = max(0, 128 * i - 512 * c)
                            n = 512 - sc
                            diag = 4 * c <= i <= 4 * c + 3
                            sps = {}
                            for z in range(2):
                                sp = ps6.tile([P, 512], F32, tag="ps",
                                              name=f"sp{z}")
                                nc.tensor.matmul(
                                    sp[:, :n],
                                    lhsT=kT[64 * z:64 * z + 64, jt,
                                            P * i:P * (i + 1)],
                                    rhs=qT[64 * z:64 * z + 64, jt,
                                           512 * c + sc:512 * (c + 1)],
                                    start=True, stop=True,
                                )
                                sps[z] = sp
                            for z, h_ in enumerate(pair):
                                e = epool.tile([P, 512], BF16, tag="e")
                                nc.scalar.activation(e[:, :n], sps[z][:, :n],
                                                     AF.Exp)
                                if diag:
                                    # masked (strictly-future) entries -> 1:
                                    # zero them in e (gpsimd), and add their
                                    # constant colsum-of-V via one extra MM.
                                    nc.gpsimd.tensor_tensor(
                                        e[:, 0:P], e[:, 0:P], tri, op=ALU.mult)
                                    nc.tensor.matmul(
                                        yaugs[z][:, sc:sc + P],
                                        lhsT=v_aug[:, i, h_, :],
                                        rhs=itri,
                                        start=False, stop=False,
                                        skip_group_check=True,
                                    )
                                nc.tensor.matmul(
                                    yaugs[z][:, sc:512],
                                    lhsT=v_aug[:, i, h_, :],
                                    rhs=e[:, :n],
                                    start=False,
                                    stop=(idx == len(ilist) - 1),
                                    skip_group_check=True,
                                )
                            for _ in range(nq):
                                if qsteps:
                                    qsteps.pop(0)()
                        while qsteps:
                            qsteps.pop(0)()
                        for z, h_ in enumerate(pair):
                            ya_sb = small.tile([HD + 1, 512], F32, tag="ya")
                            nc.vector.tensor_copy(out=ya_sb, in_=yaugs[z])
                            for j2 in range(4):
                                tb = 4 * c + j2
                                yt = ps6.tile([P, 512], F32, tag="ps",
                                              name="yt")[:, 0:P]
                                nc.tensor.transpose(
                                    yt[:, :HD + 1],
                                    ya_sb[:, P * j2:P * (j2 + 1)],
                                    ident[:HD + 1, :HD + 1],
                                )
                                rden = small.tile([P, 1], F32, tag="rden")
                                nc.vector.reciprocal(rden, yt[:, HD:HD + 1])
                                nc.vector.tensor_scalar_mul(
                                    stg[:, tb, HD * z:HD * (z + 1)],
                                    yt[:, 0:HD],
                                    rden,
                                )
                        # stage the finished row half of piece jt (vector)
                        nc.vector.tensor_scalar_mul(
                            stg2[:, 4 * c:4 * c + 4, 0:128],
                            stg[:, 4 * c:4 * c + 4, :], sel_sb[:, 0:1])
                        nc.vector.tensor_scalar_mul(
                            stg2[:, 4 * c:4 * c + 4, 128:256],
                            stg[:, 4 * c:4 * c + 4, :], sel_sb[:, 1:2])
                        nc.sync.dma_start(
                            cc_in[jt].rearrange(
                                "s (rr p) w -> p (s rr) w",
                                p=P)[:, 4 * c:4 * c + 4, :],
                            stg2[:, 4 * c:4 * c + 4, :],
                        )
                    nc.gpsimd.collective_compute(
                        "ReduceScatter", ALU.add,
                        ins=[cc_in[jt][:]], outs=[cc_out[jt][:]],
                        replica_groups=REPLICA_GROUPS,
                    )

                warm(20)

                # ---- Exchange tail: receive all pieces, add into the
                # resident fp32 x, then LN2 stats.  Strictly after the jt
                # loop so no engine queue blocks on an RS mid-attention.
                zps = []
                for jt in range(4):
                    zp = zpool.tile([P, 4, 2, P], FP8, tag="zp", name=f"zp{jt}")
                    zps.append(zp)
                    for r in range(2):
                        nc.gpsimd.dma_start(
                            zp[:, :, r, :],
                            cc_out[jt].rearrange(
                                "(o p) (r w) -> p o r w", p=P, r=2)[:, :, r, :],
                        )
                for jt in range(4):
                    for r in range(2):
                        nc.vector.tensor_tensor(
                            x_own_sb.rearrange("p o (r g w) -> p o r g w",
                                               r=2, g=4)[:, :, r, jt, :],
                            x_own_sb.rearrange("p o (r g w) -> p o r g w",
                                               r=2, g=4)[:, :, r, jt, :],
                            zps[jt][:, :, r, :],
                            op=ALU.add,
                        )
                for tb in range(4):
                    nc.vector.bn_stats(stats2[:, tb, 0, :],
                                       x_own_sb[:, tb, 0:512])
                    nc.vector.bn_stats(stats2[:, tb, 1, :],
                                       x_own_sb[:, tb, 512:1024])

            # ---- Phase 3: LN2 + MLP on own rows ----
            with tc.tile_pool(name="mlp_big", bufs=1) as mbig, \
                 tc.tile_pool(name="ln2", bufs=2) as ln2, \
                 tc.tile_pool(name="ps_mm", bufs=3, space="PSUM") as ps_mm, \
                 tc.tile_pool(name="ps_tr2", bufs=2, space="PSUM") as ps_tr2:

                h2T = mbig.tile([P, 8, TO], BF16)
                gT = mbig.tile([P, 32, TO], BF16)

                for tb in range(4):
                    mv = ln2.tile([P, 2], F32, tag="mv2")
                    nc.vector.bn_aggr(mv, stats2[:, tb, :, :])
                    sq = ln2.tile([P, 1], F32, tag="sq2")
                    nc.scalar.activation(sq, mv[:, 1:2], AF.Sqrt,
                                         bias=eps_t[:, 0:1])
                    rstd = ln2.tile([P, 1], F32, tag="rstd2")
                    nc.vector.reciprocal(rstd, sq)
                    h2 = ln2.tile([P, H], F32, tag="h2")
                    nc.vector.tensor_scalar(
                        h2, x_own_sb[:, tb, :], mv[:, 0:1], rstd,
                        ALU.subtract, ALU.mult,
                    )
                    for hi in range(8):
                        pt = ps_tr2.tile([P, P], F32, tag="tr2")
                        nc.tensor.transpose(pt, h2[:, hi * P:(hi + 1) * P],
                                            ident)
                        nc.scalar.copy(
                            out=h2T[:, hi, tb * P:(tb + 1) * P], in_=pt)

                # MLP1: FF chunk outer (each w1 chunk used once; the pool's
                # double-buffering prefetches the next chunk just-in-time).
                for wc in range(8):
                    for ft in range(4):
                        f = wc * 4 + ft
                        for tbc in range(2):
                            ps = ps_mm.tile([P, 512], F32, tag="mm",
                                            name="psg")[:, :256]
                            for hi in range(8):
                                nc.tensor.matmul(
                                    ps,
                                    lhsT=w1c[wc][:, hi, ft * P:(ft + 1) * P],
                                    rhs=h2T[:, hi, 256 * tbc:256 * (tbc + 1)],
                                    start=(hi == 0), stop=(hi == 7),
                                )
                            nc.scalar.activation(
                                gT[:, f, 256 * tbc:256 * (tbc + 1)],
                                ps, AF.Gelu,
                            )
                    if wc == 1:
                        for i in range(2, 8):
                            w1_dma(nc.sync, i)

                out_r = out.rearrange("(o p) f -> p o f", p=P)
                for tb in range(4):
                    for ch in range(2):
                        ps = ps_mm.tile([P, 512], F32, tag="mm")
                        for ft in range(32):
                            nc.tensor.matmul(
                                ps,
                                lhsT=gT[:, ft, tb * P:(tb + 1) * P],
                                rhs=w2_sb[:, ft, ch * 512:(ch + 1) * 512],
                                start=(ft == 0), stop=(ft == 31),
                            )
                        nc.vector.tensor_tensor(
                            x_own_sb[:, tb, ch * 512:(ch + 1) * 512],
                            x_own_sb[:, tb, ch * 512:(ch + 1) * 512],
                            ps, op=ALU.add,
                        )
                    nc.sync.dma_start(out_r[:, tb, :], x_own_sb[:, tb, :])

    nc.compile()
    return nc


def kernel(**inputs):
    """Full-input / full-output entry point.  See module docstring."""
    if "nc" not in _CACHE:
        _CACHE["nc"] = _build_program()
    nc = _CACHE["nc"]

    E4M3 = ml_dtypes.float8_e4m3

    def q8(a):
        return np.clip(np.asarray(a, np.float32), -240, 240).astype(E4M3)

    x = np.asarray(inputs["x"], np.float32)
    scale = 1.0 / np.sqrt(HD)
    wq_np = q8(np.asarray(inputs["Wq"], np.float32) * (scale * WS))
    wk_np = q8(np.asarray(inputs["Wk"], np.float32) * WS)
    wv_np = q8(np.asarray(inputs["Wv"], np.float32) * WS)
    w1_np = np.asarray(inputs["W1"], np.float32).astype(ml_dtypes.bfloat16)
    w2_np = np.asarray(inputs["W2"], np.float32).astype(ml_dtypes.bfloat16)
    bind_np = np.kron(np.eye(8, dtype=np.float32),
                      np.ones((1, P), np.float32)).astype(ml_dtypes.bfloat16)

    in_maps = []
    for c in range(8):
        b, half = c // 2, c % 2
        cols = slice(half * 512, (half + 1) * 512)
        in_maps.append({
            "x_full": np.ascontiguousarray(x[b]),
            "x_own": np.ascontiguousarray(x[b, half * TO:(half + 1) * TO]),
            "wq": np.ascontiguousarray(wq_np[:, cols]),
            "wk": np.ascontiguousarray(wk_np[:, cols]),
            "wv": np.ascontiguousarray(wv_np[:, cols]),
            "w1": w1_np,
            "w2": w2_np,
            "sel": np.array([[1.0, 0.0]] if half == 0 else [[0.0, 1.0]],
                            np.float32),
            "bind": bind_np,
        })

    res = run_bass_kernel_spmd(nc, in_maps, core_ids=list(range(8)))
    _CACHE["last_results"] = res

    out = np.empty((B, T, H), np.float32)
    for c in range(8):
        b, half = c // 2, c % 2
        out[b, half * TO:(half + 1) * TO] = res.results[c]["out"]
    return out


# revision 20
# speedup vs baseline: 1.0192x; 1.0192x over previous
"""Trainium2 Bass kernel for nn_MemBlock (dense transformer block).

Reference computation (B=4, T=1024, H=1024, K=16 heads, hd=64):
    h  = LN(x);  q,k,v = h@Wq, h@Wk, h@Wv  (per-head split)
    s  = q k^T / sqrt(hd);  masked (future) positions FILLED with 1e-9 (not -inf)
    a  = softmax(s);  y = a v;  x = x + y
    h2 = LN(x);  out = x + gelu(h2@W1)@W2

Key numerical fact exploited: in fp32, exp(1e-9) == 1.0 exactly, so every
"masked" (strictly-future) position carries softmax weight exp(0)=1.  A fully
masked 128x128 score block contributes plain column-sums of V to the
numerator and a count to the denominator -- folded into the attention-value
accumulation as one extra tiny matmul (suffix^T x block-indicator) per
(head, column-half).  Only lower-triangular blocks of the score matrix are
computed; the diagonal block is repaired multiplicatively on the exp output
(e = e*tri + (1-tri), on the otherwise-idle gpsimd engine) so masked entries
carry weight exactly 1, matching the reference.

Sharding (8 cores, SPMD): core c handles batch b=c//2 and head-half h=c%2:
attention over heads [8h, 8h+8) for ALL T rows, then a pairwise exchange
hands each core its own T-row half of the full-width attention output, and
each core runs LN2 + the full-weight MLP on its 512 own rows.

v7 schedule notes:
  - Engine queues are in-order, so every op is pinned to an engine chosen so
    that no queue ever head-of-line-blocks on a slower producer: scalar does
    ONLY exps during attention; vector does masks/finalize/copies; gpsimd
    (idle otherwise) repairs the diagonal blocks; the next pair's q/k fp8
    DoubleRow chains are drip-fed INTO the score/AV loop one matmul at a
    time so the PE never builds a wall in front of the next score block.
  - LN1 is software-pipelined one full stage (stats of tile t+1 issue before
    the rstd/normalize of tile t) to hide the vector->scalar->vector
    round-trip; dummy "warm" matmuls keep the PE HAM clock at 2.4 GHz
    through the PE-sparse LN1 and exchange windows.
  - The y exchange is 4 fp8 pieces (one per head pair) ReduceScatter'd as
    soon as each pair finishes; receives/adds/LN2-stats run strictly after
    the attention loop so no engine queue waits on a collective mid-flight.
  - q/k/v projections are fp8e4 DoubleRow (weights prescaled x16 on the
    host, 1/16 folded into the PSUM->SBUF copies); scores/AV stay bf16.
"""

import numpy as np
import ml_dtypes

import concourse.bass as bass
import concourse.tile as tile
from concourse import bacc, mybir
from concourse.bass_utils import run_bass_kernel_spmd
from concourse.masks import make_identity, make_upper_triangular

F32 = mybir.dt.float32
BF16 = mybir.dt.bfloat16
FP8 = mybir.dt.float8e4
AF = mybir.ActivationFunctionType
ALU = mybir.AluOpType
DR = mybir.MatmulPerfMode.DoubleRow

B, T, H, NK, HD = 4, 1024, 1024, 16, 64
NHC = 8          # heads per core
TO = 512         # own rows per core
FF = 4 * H       # 4096
P = 128
EPS = 1e-5
WS = 16.0        # fp8 weight prescale

REPLICA_GROUPS = [[0, 1], [2, 3], [4, 5], [6, 7]]

_CACHE = {}


def _build_program():
    nc = bacc.Bacc("TRN2", target_bir_lowering=False, debug=False, num_devices=8)

    x_full = nc.dram_tensor("x_full", [T, H], F32, kind="ExternalInput").ap()
    x_own = nc.dram_tensor("x_own", [TO, H], F32, kind="ExternalInput").ap()
    wq = nc.dram_tensor("wq", [H, NHC * HD], FP8, kind="ExternalInput").ap()
    wk = nc.dram_tensor("wk", [H, NHC * HD], FP8, kind="ExternalInput").ap()
    wv = nc.dram_tensor("wv", [H, NHC * HD], FP8, kind="ExternalInput").ap()
    w1 = nc.dram_tensor("w1", [H, FF], BF16, kind="ExternalInput").ap()
    w2 = nc.dram_tensor("w2", [FF, H], BF16, kind="ExternalInput").ap()
    sel = nc.dram_tensor("sel", [1, 2], F32, kind="ExternalInput").ap()
    bind = nc.dram_tensor("bind", [8, T], BF16, kind="ExternalInput").ap()
    out = nc.dram_tensor("out", [TO, H], F32, kind="ExternalOutput").ap()

    cc_wu_in = nc.dram_tensor("cc_wu_in", [2, 16], BF16)
    cc_wu_out = nc.dram_tensor("cc_wu_out", [1, 16], BF16)
    cc_in = [nc.dram_tensor(f"cc_in{p}", [2, TO, 256], BF16) for p in range(4)]
    cc_out = [nc.dram_tensor(f"cc_out{p}", [TO, 256], BF16) for p in range(4)]

    with tile.TileContext(nc) as tc:
        with tc.tile_pool(name="consts", bufs=1) as consts, \
             tc.tile_pool(name="persist", bufs=1) as persist, \
             tc.tile_pool(name="w1pool", bufs=2) as w1pool:

            ident = consts.tile([P, P], F32)
            make_identity(nc, ident)
            tri = consts.tile([P, P], BF16)   # tri[p,t] = 1 if p <= t else 0
            make_upper_triangular(nc, tri, val=1.0, diag=True)
            itri = consts.tile([P, P], BF16)  # 1 - tri (strictly-lower)
            nc.vector.memset(itri, 1.0)
            nc.vector.tensor_tensor(itri, itri, tri, op=ALU.subtract)
            eps_t = consts.tile([P, 1], F32)
            nc.vector.memset(eps_t, EPS)
            # ind[p, i, j] = 1 if i > j else 0 (suffix-of-blocks indicator)
            ind = consts.tile([P, 8, 8], BF16)
            nc.vector.memset(ind, 0.0)
            for i in range(1, 8):
                nc.vector.memset(ind[:, i, 0:i], 1.0)
            # blockind[j, q] = 1 if q//128 == j (suffix broadcast MM rhs)
            blockind = consts.tile([8, T], BF16)
            nc.gpsimd.dma_start(out=blockind, in_=bind)
            dmy = consts.tile([P, 512], BF16)
            nc.vector.memset(dmy, 1.0)
            sel_sb = consts.tile([P, 2], F32)
            nc.gpsimd.dma_start(
                out=sel_sb,
                in_=bass.AP(tensor=sel.tensor, offset=0, ap=[[0, P], [1, 2]]),
            )

            x_own_sb = persist.tile([P, 4, H], F32)  # becomes z = x+y, then out
            w2_sb = persist.tile([P, 32, H], BF16)
            stats2 = persist.tile([P, 4, 2, 6], F32)

            with tc.tile_pool(name="attn_big", bufs=1) as big, \
                 tc.tile_pool(name="epool", bufs=4) as epool, \
                 tc.tile_pool(name="small", bufs=2) as small, \
                 tc.tile_pool(name="stgpool", bufs=2) as stgpool, \
                 tc.tile_pool(name="zpool", bufs=4) as zpool, \
                 tc.tile_pool(name="ln", bufs=3) as ln, \
                 tc.tile_pool(name="ps6", bufs=4, space="PSUM") as ps6, \
                 tc.tile_pool(name="ps_yaug", bufs=2, space="PSUM") as ps_yaug, \
                 tc.tile_pool(name="ps_qkv", bufs=2, space="PSUM") as ps_qkv:

                hT8 = big.tile([P, 8, T], FP8)
                qT = big.tile([P, 4, T], BF16)
                kT = big.tile([P, 4, T], BF16)
                v_aug = big.tile([P, 8, NHC, HD + 1], BF16)
                wq_sb = big.tile([P, 8, NHC * HD], FP8)
                wk_sb = big.tile([P, 8, NHC * HD], FP8)
                wv_sb = big.tile([P, 8, NHC * HD], FP8)
                nc.gpsimd.dma_start(out=wv_sb, in_=wv.rearrange("(o p) j -> p o j", p=P))
                nc.gpsimd.dma_start(out=wq_sb, in_=wq.rearrange("(o p) j -> p o j", p=P))
                nc.gpsimd.dma_start(out=wk_sb, in_=wk.rearrange("(o p) j -> p o j", p=P))
                nc.gpsimd.dma_start(x_own_sb, x_own.rearrange("(o p) f -> p o f", p=P))
                # dummy collective to absorb the CC-stream warmup delay
                nc.gpsimd.collective_compute(
                    "ReduceScatter", ALU.add,
                    ins=[cc_wu_in[:]], outs=[cc_wu_out[:]],
                    replica_groups=REPLICA_GROUPS,
                )

                def warm(nmm):
                    """No-dep matmuls keeping the PE HAM clock at 2.4 GHz
                    through PE-sparse stretches."""
                    wp = ps6.tile([P, 512], F32, tag="ps", name="warm")
                    for _ in range(nmm):
                        nc.tensor.matmul(
                            wp, lhsT=dmy[:, 0:P], rhs=dmy,
                            start=True, stop=True, skip_group_check=True,
                        )

                warm(12)

                # ---- Phase 1: LN1, software-pipelined one stage ----
                n_sc_cp = 6  # hT8 cast-copies on scalar (rest on vector)

                def ln1_a(tt, st):
                    xt = ln.tile([P, H], F32, tag="xt", name="xt")
                    st['xt'] = xt
                    nc.sync.dma_start(xt, x_full[tt * P:(tt + 1) * P, :])
                    stats = ln.tile([P, 2, 6], F32, tag="stats", name="stats")
                    st['stats'] = stats
                    nc.vector.bn_stats(stats[:, 0, :], xt[:, 0:512])
                    nc.vector.bn_stats(stats[:, 1, :], xt[:, 512:1024])
                    mv = ln.tile([P, 2], F32, tag="mv", name="mv")
                    st['mv'] = mv
                    nc.vector.bn_aggr(mv, stats)
                    sq = ln.tile([P, 1], F32, tag="sq", name="sq")
                    st['sq'] = sq
                    nc.scalar.activation(sq, mv[:, 1:2], AF.Sqrt,
                                         bias=eps_t[:, 0:1])

                def ln1_b(tt, st):
                    rstd = ln.tile([P, 1], F32, tag="rstd")
                    nc.vector.reciprocal(rstd, st['sq'])
                    h = ln.tile([P, H], F32, tag="h")
                    nc.vector.tensor_scalar(
                        h, st['xt'], st['mv'][:, 0:1], rstd,
                        ALU.subtract, ALU.mult,
                    )
                    for hi in range(8):
                        pt = ps6.tile([P, 512], F32, tag="ps", name="trp")[:, 0:P]
                        nc.tensor.transpose(pt, h[:, hi * P:(hi + 1) * P], ident)
                        if hi < n_sc_cp:
                            nc.scalar.copy(
                                out=hT8[:, hi, tt * P:(tt + 1) * P], in_=pt)
                        else:
                            nc.vector.tensor_copy(
                                out=hT8[:, hi, tt * P:(tt + 1) * P], in_=pt)
                    ps = ps_qkv.tile([P, 512], F32, tag="qkv")
                    for kk in range(4):
                        nc.tensor.matmul(
                            ps,
                            lhsT=hT8[:, 2 * kk:2 * kk + 2, tt * P:(tt + 1) * P],
                            rhs=wv_sb[:, 2 * kk:2 * kk + 2, :],
                            start=(kk == 0), stop=(kk == 3), perf_mode=DR,
                        )
                    nc.scalar.mul(
                        v_aug[:, tt, :, 0:HD],
                        ps.rearrange("p (h d) -> p h d", h=NHC),
                        1.0 / WS,
                    )
                    warm(5)

                sts = [dict() for _ in range(8)]
                ln1_a(0, sts[0])
                for tt in range(8):
                    if tt < 7:
                        ln1_a(tt + 1, sts[tt + 1])
                    ln1_b(tt, sts[tt])
                nc.vector.memset(v_aug[:, :, :, HD:HD + 1], 1.0)

                # Early weight DMAs on the scalar queue; late w1 chunks go
                # just-in-time on sync during MLP1.
                w1c = [w1pool.tile([P, 8, 512], BF16, tag="w1c", name=f"w1c{i}")
                       for i in range(8)]

                def w1_dma(eng, i):
                    eng.dma_start(
                        w1c[i],
                        w1[:, i * 512:(i + 1) * 512].rearrange(
                            "(o p) f -> p o f", p=P),
                    )

                w1_dma(nc.scalar, 0)
                w1_dma(nc.scalar, 1)
                nc.scalar.dma_start(w2_sb, w2.rearrange("(o p) n -> p o n", p=P))

                def qk_steps(jt, ch):
                    """One-matmul-at-a-time issue steps for pair jt's q/k
                    DoubleRow chains over T-column half ch."""
                    tiles = {}
                    steps = []

                    def mk(dst, w_sb, kk, name):
                        def f():
                            if kk == 0:
                                tiles[name] = ps_qkv.tile(
                                    [P, 512], F32, tag="qkv", name=name)
                            nc.tensor.matmul(
                                tiles[name],
                                lhsT=w_sb[:, 2 * kk:2 * kk + 2,
                                          jt * P:(jt + 1) * P],
                                rhs=hT8[:, 2 * kk:2 * kk + 2,
                                        ch * 512:(ch + 1) * 512],
                                start=(kk == 0), stop=(kk == 3), perf_mode=DR,
                            )
                            if kk == 3:
                                nc.vector.tensor_scalar_mul(
                                    dst[:, jt, ch * 512:(ch + 1) * 512],
                                    tiles[name], 1.0 / WS)
                        return f

                    for kk in range(4):
                        steps.append(mk(qT, wq_sb, kk, "psq"))
                        steps.append(mk(kT, wk_sb, kk, "psk"))
                    return steps

                for st in qk_steps(0, 0) + qk_steps(0, 1):
                    st()

                # ---- Phase 2: attention per head pair ----
                for jt in range(4):
                    pair = (2 * jt, 2 * jt + 1)

                    # sufT[j, d] = sum_{i>j} colsum(V_aug_i)[d]: [8, 65]/head
                    sufp = ps6.tile([P, 512], F32, tag="ps", name="sufp")
                    for z, h_ in enumerate(pair):
                        for i in range(1, 8):
                            nc.tensor.matmul(
                                sufp[0:8, 65 * z:65 * z + 65],
                                lhsT=ind[:, i, :],
                                rhs=v_aug[:, i, h_, :],
                                start=(i == 1), stop=(i == 7),
                                skip_group_check=True,
                            )
                    sufT_sb = small.tile([8, 130], BF16, tag="sufT")
                    nc.vector.tensor_copy(out=sufT_sb, in_=sufp[0:8, 0:130])

                    stg = stgpool.tile([P, 8, P], BF16, tag="stg")
                    stg2 = stgpool.tile([P, 8, 256], BF16, tag="stg2")

                    for c in range(2):
                        qsteps = qk_steps(jt + 1, c) if jt < 3 else []
                        yaugs = [
                            ps_yaug.tile([HD + 1, 512], F32, tag="yaug",
                                         name=f"yaug{z}")
                            for z in range(2)
                        ]
                        for z in range(2):
                            nc.tensor.matmul(
                                yaugs[z],
                                lhsT=sufT_sb[0:8, 65 * z:65 * z + 65],
                                rhs=blockind[0:8, 512 * c:512 * (c + 1)],
                                start=True, stop=False, skip_group_check=True,
                            )
                        # non-diagonal key blocks first: their exps flow with
                        # no repair op in the score->exp->AV chain
                        ilist = [i for i in range(8)
                                 if 512 * (c + 1) - 128 * i > 0]
                        ilist = ([i for i in ilist
                                  if not 4 * c <= i <= 4 * c + 3]
                                 + [i for i in ilist if 4 * c <= i <= 4 * c + 3])
                        nq = ((len(qsteps) + len(ilist) - 1) // len(ilist)
                              if qsteps else 0)
                        for idx, i in enumerate(ilist):
                            sc = max(0, 128 * i - 512 * c)
                            n = 512 - sc
                            diag = 4 * c <= i <= 4 * c + 3
                            sps = {}
                            for z in range(2):
                                sp = ps6.tile([P, 512], F32, tag="ps",
                                              name=f"sp{z}")
                                nc.tensor.matmul(
                                    sp[:, :n],
                                    lhsT=kT[64 * z:64 * z + 64, jt,
                                            P * i:P * (i + 1)],
                                    rhs=qT[64 * z:64 * z + 64, jt,
                                           512 * c + sc:512 * (c + 1)],
                                    start=True, stop=True,
                                )
                                sps[z] = sp
                            for z, h_ in enumerate(pair):
                                e = epool.tile([P, 512], BF16, tag="e")
                                nc.scalar.activation(e[:, :n], sps[z][:, :n],
                                                     AF.Exp)
                                if diag:
                                    # masked (strictly-future) entries -> 1:
                                    # zero them in e (gpsimd), and add their
                                    # constant colsum-of-V via one extra MM.
                                    nc.gpsimd.tensor_tensor(
                                        e[:, 0:P], e[:, 0:P], tri, op=ALU.mult)
                                    nc.tensor.matmul(
                                        yaugs[z][:, sc:sc + P],
                                        lhsT=v_aug[:, i, h_, :],
                                        rhs=itri,
                                        start=False, stop=False,
                                        skip_group_check=True,
                                    )
                                nc.tensor.matmul(
                                    yaugs[z][:, sc:512],
                                    lhsT=v_aug[:, i, h_, :],
                                    rhs=e[:, :n],
                                    start=False,
                                    stop=(idx == len(ilist) - 1),
                                    skip_group_check=True,
                                )
                            for _ in range(nq):
                                if qsteps:
                                    qsteps.pop(0)()
                        while qsteps:
                            qsteps.pop(0)()
                        for z, h_ in enumerate(pair):
                            ya_sb = small.tile([HD + 1, 512], F32, tag="ya")
                            nc.vector.tensor_copy(out=ya_sb, in_=yaugs[z])
                            for j2 in range(4):
                                tb = 4 * c + j2
                                yt = ps6.tile([P, 512], F32, tag="ps",
                                              name="yt")[:, 0:P]
                                nc.tensor.transpose(
                                    yt[:, :HD + 1],
                                    ya_sb[:, P * j2:P * (j2 + 1)],
                                    ident[:HD + 1, :HD + 1],
                                )
                                rden = small.tile([P, 1], F32, tag="rden")
                                nc.vector.reciprocal(rden, yt[:, HD:HD + 1])
                                nc.vector.tensor_scalar_mul(
                                    stg[:, tb, HD * z:HD * (z + 1)],
                                    yt[:, 0:HD],
                                    rden,
                                )
                        # stage the finished row half of piece jt (vector)
                        nc.vector.tensor_scalar_mul(
                            stg2[:, 4 * c:4 * c + 4, 0:128],
                            stg[:, 4 * c:4 * c + 4, :], sel_sb[:, 0:1])
                        nc.vector.tensor_scalar_mul(
                            stg2[:, 4 * c:4 * c + 4, 128:256],
                            stg[:, 4 * c:4 * c + 4, :], sel_sb[:, 1:2])
                        nc.sync.dma_start(
                            cc_in[jt].rearrange(
                                "s (rr p) w -> p (s rr) w",
                                p=P)[:, 4 * c:4 * c + 4, :],
                            stg2[:, 4 * c:4 * c + 4, :],
                        )
                    nc.gpsimd.collective_compute(
                        "ReduceScatter", ALU.add,
                        ins=[cc_in[jt][:]], outs=[cc_out[jt][:]],
                        replica_groups=REPLICA_GROUPS,
                    )

                warm(20)

                # ---- Exchange tail: receive all pieces, add into the
                # resident fp32 x, then LN2 stats.  Strictly after the jt
                # loop so no engine queue blocks on an RS mid-attention.
                zps = []
                for jt in range(4):
                    zp = zpool.tile([P, 4, 2, P], BF16, tag="zp", name=f"zp{jt}")
                    zps.append(zp)
                    # scheduling fence: a tiny write that depends on the LAST
                    # pair's staging data keeps the recv DMA (WAW on zp) from
                    # being scheduled into the attention-era gpsimd stream,
                    # where it would head-of-line-block on the RS.
                    nc.vector.tensor_copy(out=zp[0:1, 0, :, 0:2],
                                          in_=stg[0:1, 0, 0:4].rearrange(
                                              "p (r w) -> p r w", r=2))
                    for r in range(2):
                        nc.gpsimd.dma_start(
                            zp[:, :, r, :],
                            cc_out[jt].rearrange(
                                "(o p) (r w) -> p o r w", p=P, r=2)[:, :, r, :],
                        )
                for jt in range(4):
                    for r in range(2):
                        nc.vector.tensor_tensor(
                            x_own_sb.rearrange("p o (r g w) -> p o r g w",
                                               r=2, g=4)[:, :, r, jt, :],
                            x_own_sb.rearrange("p o (r g w) -> p o r g w",
                                               r=2, g=4)[:, :, r, jt, :],
                            zps[jt][:, :, r, :],
                            op=ALU.add,
                        )
                for tb in range(4):
                    nc.vector.bn_stats(stats2[:, tb, 0, :],
                                       x_own_sb[:, tb, 0:512])
                    nc.vector.bn_stats(stats2[:, tb, 1, :],
                                       x_own_sb[:, tb, 512:1024])

            # ---- Phase 3: LN2 + MLP on own rows ----
            with tc.tile_pool(name="mlp_big", bufs=1) as mbig, \
                 tc.tile_pool(name="ln2", bufs=2) as ln2, \
                 tc.tile_pool(name="ps_mm", bufs=3, space="PSUM") as ps_mm, \
                 tc.tile_pool(name="ps_tr2", bufs=2, space="PSUM") as ps_tr2:

                h2T = mbig.tile([P, 8, TO], BF16)
                gT = mbig.tile([P, 32, TO], BF16)

                for tb in range(4):
                    mv = ln2.tile([P, 2], F32, tag="mv2")
                    nc.vector.bn_aggr(mv, stats2[:, tb, :, :])
                    sq = ln2.tile([P, 1], F32, tag="sq2")
                    nc.scalar.activation(sq, mv[:, 1:2], AF.Sqrt,
                                         bias=eps_t[:, 0:1])
                    rstd = ln2.tile([P, 1], F32, tag="rstd2")
                    nc.vector.reciprocal(rstd, sq)
                    h2 = ln2.tile([P, H], F32, tag="h2")
                    nc.vector.tensor_scalar(
                        h2, x_own_sb[:, tb, :], mv[:, 0:1], rstd,
                        ALU.subtract, ALU.mult,
                    )
                    for hi in range(8):
                        pt = ps_tr2.tile([P, P], F32, tag="tr2")
                        nc.tensor.transpose(pt, h2[:, hi * P:(hi + 1) * P],
                                            ident)
                        nc.scalar.copy(
                            out=h2T[:, hi, tb * P:(tb + 1) * P], in_=pt)

                # MLP1: FF chunk outer (each w1 chunk used once; the pool's
                # double-buffering prefetches the next chunk just-in-time).
                for wc in range(8):
                    for ft in range(4):
                        f = wc * 4 + ft
                        for tbc in range(2):
                            ps = ps_mm.tile([P, 512], F32, tag="mm",
                                            name="psg")[:, :256]
                            for hi in range(8):
                                nc.tensor.matmul(
                                    ps,
                                    lhsT=w1c[wc][:, hi, ft * P:(ft + 1) * P],
                                    rhs=h2T[:, hi, 256 * tbc:256 * (tbc + 1)],
                                    start=(hi == 0), stop=(hi == 7),
                                )
                            nc.scalar.activation(
                                gT[:, f, 256 * tbc:256 * (tbc + 1)],
                                ps, AF.Gelu,
                            )
                    if wc == 1:
                        for i in range(2, 8):
                            w1_dma(nc.sync, i)

                out_r = out.rearrange("(o p) f -> p o f", p=P)
                for tb in range(4):
                    for ch in range(2):
                        ps = ps_mm.tile([P, 512], F32, tag="mm")
                        for ft in range(32):
                            nc.tensor.matmul(
                                ps,
                                lhsT=gT[:, ft, tb * P:(tb + 1) * P],
                                rhs=w2_sb[:, ft, ch * 512:(ch + 1) * 512],
                                start=(ft == 0), stop=(ft == 31),
                            )
                        nc.vector.tensor_tensor(
                            x_own_sb[:, tb, ch * 512:(ch + 1) * 512],
                            x_own_sb[:, tb, ch * 512:(ch + 1) * 512],
                            ps, op=ALU.add,
                        )
                    nc.sync.dma_start(out_r[:, tb, :], x_own_sb[:, tb, :])

    nc.compile()
    return nc


def kernel(**inputs):
    """Full-input / full-output entry point.  See module docstring."""
    if "nc" not in _CACHE:
        _CACHE["nc"] = _build_program()
    nc = _CACHE["nc"]

    E4M3 = ml_dtypes.float8_e4m3

    def q8(a):
        return np.clip(np.asarray(a, np.float32), -240, 240).astype(E4M3)

    x = np.asarray(inputs["x"], np.float32)
    scale = 1.0 / np.sqrt(HD)
    wq_np = q8(np.asarray(inputs["Wq"], np.float32) * (scale * WS))
    wk_np = q8(np.asarray(inputs["Wk"], np.float32) * WS)
    wv_np = q8(np.asarray(inputs["Wv"], np.float32) * WS)
    w1_np = np.asarray(inputs["W1"], np.float32).astype(ml_dtypes.bfloat16)
    w2_np = np.asarray(inputs["W2"], np.float32).astype(ml_dtypes.bfloat16)
    bind_np = np.kron(np.eye(8, dtype=np.float32),
                      np.ones((1, P), np.float32)).astype(ml_dtypes.bfloat16)

    in_maps = []
    for c in range(8):
        b, half = c // 2, c % 2
        cols = slice(half * 512, (half + 1) * 512)
        in_maps.append({
            "x_full": np.ascontiguousarray(x[b]),
            "x_own": np.ascontiguousarray(x[b, half * TO:(half + 1) * TO]),
            "wq": np.ascontiguousarray(wq_np[:, cols]),
            "wk": np.ascontiguousarray(wk_np[:, cols]),
            "wv": np.ascontiguousarray(wv_np[:, cols]),
            "w1": w1_np,
            "w2": w2_np,
            "sel": np.array([[1.0, 0.0]] if half == 0 else [[0.0, 1.0]],
                            np.float32),
            "bind": bind_np,
        })

    res = run_bass_kernel_spmd(nc, in_maps, core_ids=list(range(8)))
    _CACHE["last_results"] = res

    out = np.empty((B, T, H), np.float32)
    for c in range(8):
        b, half = c // 2, c % 2
        out[b, half * TO:(half + 1) * TO] = res.results[c]["out"]
    return out
